# revision 1
# baseline (speedup 1.0000x reference)
"""TransformerXL attention (AttentionXL) Bass kernel for Trainium2, 8 NeuronCores.

Sharding: pure data-parallel over batch (BS=8 -> 1 batch element per core).
All weights replicated per core; no collectives.

Transposed-score pipeline: attention scores live as [key j, query i] so the
attention matrix never needs a PE transpose (v1 spent ~120us/core on 416 of
them and the HAM clock-gate punished the idle gaps they left):

  Host prep:  X^T, Xc^T, W_kv split, bias folds, and the whole batch-
              independent R projection R = pos_emb @ W_pos + b_pos.
  Device, stage A:  KT [hd, j], QuT/QvT [hd, i] (+bias), VA [j, 65-col slots
              per head: V_h | ones] - the ones column makes the AV matmul
              also emit the softmax normalizer Z as PSUM row 64.
  The rel-shift: P [i, m] is written to DRAM with row pitch 1025 and a +1
  pre-pad, which makes S[i, j] = P[i, 511+j-i] one CONTIGUOUS [512, 1024]
  block at offset 512; a single hardware xbar transpose-DMA per head lands
  S^T [j, i] in SBUF.  The pad slot and the sub-diagonal region carry -30000
  poison, so every causally masked position (j - i > 512) reads poison and
  exp()s to zero - no mask op ever touches the score matrix.
  Per head pair (heads 2hp/2hp+1 on PE row-groups 0-63/64-127, emitted
  adjacently so the 64-contraction score matmuls run concurrently; each
  PSUM pair tile is [P, 2, 512] so the two concurrent matmul groups sit in
  different 2KB banks - two groups in ONE bank hang the device):
    C^T [j, i] chunks (trimmed to i >= 128*(jc-4)); DVE adds S^T in PSUM;
    ScalarE exp overwrites S^T in SBUF with E; AV accumulates
    O^T_aug [65, i] = sum_jc VA^T E; 1/Z (DVE reciprocal) is broadcast to 64
    partitions by a tiny ones-column matmul and fused into the PSUM->SBUF
    normalize copy.
  The first 3 pairs' position scores are emitted between the Q and K/V
  projections so their DMA round trips and transposes hide under ~55us of
  stage-A matmuls; the pair loop then software-pipelines 3 deep.
  out[i,e] = AVT^T @ W_proj + b_out (ones-row bias matmul), fp32.
"""

import os
import sys

for _p in (
    "/root/.axon_site",
    "/root/.axon_site/_ro/trn_rl_repo",
    "/root/.axon_site/_ro/pypackages",
    "/opt/trn_rl_repo",
):
    if os.path.isdir(_p) and _p not in sys.path:
        sys.path.append(_p)

import numpy as np
import ml_dtypes

import concourse.bass as bass
import concourse.mybir as mybir
import concourse.tile as tile
from concourse.bass_utils import run_bass_kernel_spmd
from concourse.masks import make_identity

BF16 = mybir.dt.bfloat16
FP32 = mybir.dt.float32
AF = mybir.ActivationFunctionType
ALU = mybir.AluOpType
nbf16 = ml_dtypes.bfloat16

CUR, FULL, BS, DIM, H, D = 512, 1024, 8, 1024, 16, 64
PREV = FULL - CUR
SCALE = 1.0 / D**0.5
P = 128
NIB = CUR // P    # 4 query blocks
NJC = FULL // P   # 8 key chunks
NCH = DIM // P    # 8 dim chunks
NHP = H // 2      # 8 head pairs
PITCH = FULL + 1   # 1025: pad | 1024 score cols; makes the rel-shift contiguous
PITCH2 = 2 * FULL + 1  # 2049: pad | par0 row | par1 row -> ONE transpose per pair
POISON = -30000.0

_BUILT = None


def _split_multiwait(nc):
    """walrus here encodes at most ONE sync wait per TPB instruction
    (NEURON_ISA_TPB_EVENTS has a single wait slot).  Split every
    multi-wait instruction: prepend same-engine NoOps carrying the
    extra waits, keep the last wait on the instruction itself."""
    n_split = 0
    for fn in nc.m.functions:
        for blk in fn.blocks:
            insts = list(blk.instructions)
            out = []
            for ins in insts:
                si = ins.sync_info
                if si is not None and si.on_wait and len(si.on_wait) > 1:
                    waits = list(si.on_wait)
                    for w in waits[:-1]:
                        nop = mybir.InstNoOp(
                            name=f"{ins.name}-ws{n_split}",
                            engine=ins.engine,
                            sync_info=mybir.SyncInfo(on_wait=[w], on_update=[]),
                            text_hint="waitsplit",
                        )
                        out.append(nop)
                        n_split += 1
                    ins.sync_info = mybir.SyncInfo(
                        on_wait=[waits[-1]],
                        on_update=list(si.on_update or []),
                    )
                out.append(ins)
            blk.instructions = out
    return n_split


def _build(split_waits=True):
    nc = bass.Bass()

    # acts: [X^T | Xc^T | Pos^T] cols; wmats: [W_q | W_pos | W_k | W_v] cols
    acts = nc.declare_dram_parameter("acts", [DIM, FULL + CUR], BF16, isOutput=False)
    wmats = nc.declare_dram_parameter("wmats", [DIM, 3 * DIM], BF16, isOutput=False)
    rmat = nc.declare_dram_parameter("rmat", [DIM, FULL], BF16, isOutput=False)
    wproj = nc.declare_dram_parameter("wproj", [DIM, DIM], BF16, isOutput=False)
    # biases pre-laid-out on host: [p, 4*NCH] = qu | qv | k | pos chunks
    biases = nc.declare_dram_parameter("biases", [P, 4 * NCH], FP32, isOutput=False)
    bout = nc.declare_dram_parameter("bout", [DIM], BF16, isOutput=False)
    out = nc.declare_dram_parameter("out", [CUR, DIM], FP32, isOutput=True)

    with tile.TileContext(nc) as tc:
        from contextlib import ExitStack

        with ExitStack() as ctx:
            persist = ctx.enter_context(tc.tile_pool(name="persist", bufs=1))

            KT = persist.tile([P, NCH, FULL], BF16, tag="KT")
            RT = persist.tile([P, NCH, FULL], BF16, tag="RT")
            QuT = persist.tile([P, NCH, CUR], BF16, tag="QuT")
            QvT = persist.tile([P, NCH, CUR], BF16, tag="QvT")
            # per head-pair: [Va(64) | 1 | Vb(64) | 1] -> 130 cols
            VA = persist.tile([P, NJC, NHP, 130], BF16, tag="VA")
            AVT = persist.tile([P, NCH, CUR], BF16, tag="AVT")
            ones_row = persist.tile([P, P], BF16, tag="ones_row")
            ones1 = persist.tile([1, D], BF16, tag="ones1")
            bout_t = persist.tile([P, DIM], BF16, tag="bout_t")
            bias_t = persist.tile([P, 4, NCH], FP32, tag="bias_t")  # qu|qv|k|pos

            ident = persist.tile([P, P], BF16, tag="ident")
            make_identity(nc, ident)
            poison_reg = nc.gpsimd.to_reg(POISON)
            nc.vector.memset(ones_row, 0.0)
            nc.vector.memset(ones_row[0:1, :], 1.0)
            nc.vector.memset(ones1, 1.0)
            nc.vector.memset(bout_t, 0.0)
            nc.sync.dma_start(bout_t[0:1, :], bout[None, :])
            nc.sync.dma_start(bias_t, biases.rearrange("p (b c) -> p b c", b=4))

            # ---------------- Stage A + pipelined stage B ----------------
            # PSUM phasing: apsum(4) + pps(4) during projections/prologue,
            # then apsum closes and cps(4) opens for the pair loop.
            late = ctx.enter_context(tc.tile_pool(name="late", bufs=1))
            WPROJ = late.tile([P, NCH, DIM], BF16, tag="WPROJ")
            dmy2 = late.tile([P, 4], FP32, tag="dmy2")

            sb_ctx = ctx.enter_context(ExitStack())
            p_pool = sb_ctx.enter_context(tc.tile_pool(name="p_pool", bufs=2))
            s_pool = sb_ctx.enter_context(tc.tile_pool(name="s_pool", bufs=3))
            rz_pool = sb_ctx.enter_context(tc.tile_pool(name="rz_pool", bufs=2))
            nrm_pool = sb_ctx.enter_context(tc.tile_pool(name="nrm_pool", bufs=4))
            dram = sb_ctx.enter_context(tc.tile_pool(name="dram", bufs=3, space="DRAM"))
            pps = sb_ctx.enter_context(tc.tile_pool(name="pps", bufs=2, space="PSUM"))
            cps = None  # opened after stage A's psum pool closes

            def emit_P(hp):
                """Position scores for head pair hp -> sheared DRAM ->
                ONE transpose-DMA back as S^T [par, j, i] (poison masking).
                Pair-row layout [pad | par0 row | par1 row] at pitch 2049
                keeps the rel-shift contiguous across BOTH heads."""
                pd = dram.tile([CUR, PITCH2], BF16, tag="pd")
                s_t = s_pool.tile([P, 2, NJC, CUR], BF16, tag="s_t")
                for ib in range(NIB):
                    isl = slice(ib * P, (ib + 1) * P)
                    pib = p_pool.tile([P, PITCH2], BF16, tag="pib")
                    if ib < 3:
                        # pad + m < 128 of each parity: poison (masked reads)
                        nc.vector.memset(pib[:, 0:129], POISON)
                        nc.vector.memset(pib[:, 1025:1153], POISON)
                        mranges = [(128, 576), (576, 1024)]
                    else:
                        nc.vector.memset(pib[:, 0:1], POISON)
                        mranges = [(0, 512), (512, 1024)]
                    pibap = pib[:, :]
                    for q, (m0, m1) in enumerate(mranges):
                        w = m1 - m0
                        # [P, 2, 512]: each parity in its own PSUM bank --
                        # two concurrent matmul groups in ONE bank hang the
                        # device (probed), bank-split pairs are fine
                        pp = pps.tile([P, 2, 512], FP32, tag="pp")
                        for b in range(2):
                            rs = slice(b * D, (b + 1) * D)
                            nc.tensor.matmul(
                                pp[:, b, :w],
                                QvT[rs, hp, isl],
                                RT[rs, hp, m0:m1],
                                start=True, stop=True,
                            )
                        dst = bass.AP(tensor=pibap.tensor,
                                      offset=pibap.offset + 1 + m0,
                                      ap=[pibap.ap[0], [1024, 2], [1, w]])
                        ceng = nc.scalar.copy if q % 2 == 0 else nc.vector.tensor_copy
                        ceng(dst, pp[:, :, :w])
                    if ib == 3:
                        # sub-diagonal triangle m < 511-r for rows 384+u
                        tri = bass.AP(tensor=pibap.tensor,
                                      offset=pibap.offset + 1,
                                      ap=[pibap.ap[0], [1024, 2], [1, 128]])
                        nc.gpsimd.affine_select(
                            out=tri, in_=tri,
                            compare_op=ALU.is_ge,
                            fill=poison_reg,
                            base=-127,
                            channel_multiplier=1,
                            pattern=[[0, 2], [1, 128]],
                        )
                    dest = bass.AP(
                        tensor=pd.tensor,
                        offset=pd.offset + ib * P * PITCH2,
                        ap=[[PITCH2, P], [1, PITCH2]],
                    )
                    nc.sync.dma_start(dest, pib)
                src = bass.AP(
                    tensor=pd.tensor,
                    offset=pd.offset + 512,
                    ap=[[2048, 512], [1, 2048]],
                )
                nc.sync.dma_start(s_t, src, transpose=True)
                return s_t

            pending_norm = []

            def emit_CAV(hp, s_t):
                """Content scores + softmax + AV for head pair hp.
                E (exp) overwrites s_t in place.  The previous pair's deferred
                normalize flushes between the chunk and AV phases."""
                for jc in range(NJC):
                    jsl = slice(jc * P, (jc + 1) * P)
                    ilo = max(0, jc - 4) * P
                    w = CUR - ilo
                    cp = cps.tile([P, 2, 512], FP32, tag="cp")
                    for b in range(2):
                        rs = slice(b * D, (b + 1) * D)
                        nc.tensor.matmul(
                            cp[:, b, :w],
                            KT[rs, hp, jsl],
                            QuT[rs, hp, ilo:],
                            start=True, stop=False,
                        )
                    # accumulate S^T on the PE (identity matmul): frees the
                    # DVE add AND keeps TensorE busy enough to hold K=8/8
                    for b in range(2):
                        nc.tensor.matmul(
                            cp[:, b, :w],
                            ident,
                            s_t[:, b, jc, ilo:],
                            start=False, stop=True,
                        )
                    nc.scalar.activation(
                        s_t[:, :, jc, ilo:], cp[:, :, :w], AF.Exp, scale=SCALE
                    )
                flush_norm()
                for b in range(2):
                    av_t = cps.tile([P, 2, 512], FP32, tag="cp")
                    for jc in range(NJC):
                        ilo = max(0, jc - 4) * P
                        nc.tensor.matmul(
                            av_t[0:D + 1, 0, ilo:],
                            VA[:, jc, hp, 65 * b:65 * b + D + 1],
                            s_t[:, b, jc, ilo:],
                            start=(jc == 0),
                            stop=(jc == NJC - 1),
                        )
                    # copy O^T_unnorm + Z out of PSUM right away (frees the
                    # ring slot) and start the 1/Z chain on DVE; the PE-side
                    # broadcast matmul is DEFERRED until the next pair's chunk
                    # phase so it never head-of-line-blocks the PE queue
                    # waiting ~5us for the reciprocal (16 such stalls before)
                    rs = slice(b * D, (b + 1) * D)
                    av_sb = nrm_pool.tile([P, CUR], BF16, tag="av_sb")
                    nc.vector.tensor_copy(av_sb[rs, :], av_t[0:D, 0, :])
                    zrow = rz_pool.tile([1, CUR], BF16, tag="zrow")
                    nc.vector.tensor_copy(zrow, av_t[D:D + 1, 0, :])
                    rzh = nrm_pool.tile([1, CUR], BF16, tag="rzh")
                    with nc.allow_low_precision(reason="1/Z bf16; v1-class precision"):
                        nc.vector.reciprocal(rzh, zrow)
                    pending_norm.append((hp, b, av_sb, rzh))

            def flush_norm():
                while pending_norm:
                    hp, b, av_sb, rzh = pending_norm.pop(0)
                    rs = slice(b * D, (b + 1) * D)
                    bt = cps.tile([P, 2, 512], FP32, tag="cp")
                    nc.tensor.matmul(bt[rs, 1, :], ones1, rzh,
                                     start=True, stop=True, tile_position=(0, b * D))
                    bc_sb = rz_pool.tile([P, CUR], BF16, tag="bc_sb")
                    nc.scalar.copy(bc_sb[rs, :], bt[rs, 1, :])
                    nc.vector.tensor_tensor(
                        AVT[rs, hp, :], av_sb[rs, :], bc_sb[rs, :], ALU.mult
                    )

            with tc.tile_pool(name="apsum", bufs=4, space="PSUM") as apsum:
                # -- phase 1: Q projection (xc, wq) --
                with tc.tile_pool(name="ain1", bufs=1) as ain1:
                    xcw = ain1.tile([P, NCH, CUR + DIM], BF16, tag="xcw")
                    acts_r = acts.rearrange("(c p) f -> p c f", p=P)
                    wmats_r = wmats.rearrange("(c p) f -> p c f", p=P)
                    nc.sync.dma_start(xcw[:, :, 0:CUR], acts_r[:, :, FULL:FULL + CUR])
                    nc.sync.dma_start(xcw[:, :, CUR:], wmats_r[:, :, 0:DIM])
                    nc.sync.dma_start(RT, rmat.rearrange("(c p) m -> p c m", p=P))
                    xcT_t = xcw[:, :, 0:CUR]
                    wq_t = xcw[:, :, CUR:]

                    dmy = ain1.tile([P, 24], FP32, tag="dmy")
                    col = [0]
                    def _observe(eng):
                        for srcap in (xcT_t[:, 0, 0:2], wq_t[:, 0, 0:2],
                                      RT[:, 0, 0:2], bias_t[:, 0, 0:2],
                                      bout_t[:, 0:2]):
                            eng(dmy[:, col[0]: col[0] + 2], srcap)
                            col[0] = (col[0] + 2) % 24
                    _observe(nc.vector.tensor_copy)
                    _observe(nc.scalar.copy)

                    # Q^T [hd, i] then QuT/QvT with per-partition bias
                    for oc in range(NCH):
                        ps = apsum.tile([P, CUR], FP32, tag="aps")
                        for kc in range(NCH):
                            nc.tensor.matmul(
                                ps,
                                wq_t[:, kc, oc * P:(oc + 1) * P],
                                xcT_t[:, kc, :],
                                start=(kc == 0),
                                stop=(kc == NCH - 1),
                            )
                        nc.scalar.activation(
                            QuT[:, oc, :], ps, AF.Identity, bias=bias_t[:, 0, oc:oc + 1]
                        )
                        nc.scalar.activation(
                            QvT[:, oc, :], ps, AF.Identity, bias=bias_t[:, 1, oc:oc + 1]
                        )

                    # -- prologue: first PRO head pairs' position scores; their
                    # DMA round trips + transposes hide under the K/V matmuls
                    PRO = int(os.environ.get("V2_PRO", "2"))
                    s_ts = {}
                    for hp in range(PRO):
                        s_ts[hp] = emit_P(hp)

                # -- phase 2: K and V projections (x, wk, wv) --
                with tc.tile_pool(name="ain2", bufs=1) as ain2:
                    xw = ain2.tile([P, NCH, FULL + 2 * DIM], BF16, tag="xw")
                    # x split into j-halves so the K projection's first half
                    # starts after 4MB of loads instead of 6MB
                    nc.sync.dma_start(xw[:, :, 0:512], acts_r[:, :, 0:512])
                    nc.sync.dma_start(xw[:, :, FULL:FULL + DIM],
                                      wmats_r[:, :, DIM:2 * DIM])
                    nc.sync.dma_start(xw[:, :, 512:FULL], acts_r[:, :, 512:FULL])
                    nc.sync.dma_start(xw[:, :, FULL + DIM:],
                                      wmats_r[:, :, 2 * DIM:3 * DIM])
                    xT_t = xw[:, :, 0:FULL]
                    wk_t = xw[:, :, FULL:FULL + DIM]
                    wv_t = xw[:, :, FULL + DIM:]
                    dmy3 = ain2.tile([P, 8], FP32, tag="dmy3")
                    for eng in (nc.vector.tensor_copy, nc.scalar.copy):
                        eng(dmy3[:, 0:2], xT_t[:, 0, 0:2])
                        eng(dmy3[:, 2:4], wk_t[:, 0, 0:2])
                        eng(dmy3[:, 4:6], wv_t[:, 0, 0:2])

                    # K^T [hd, j]  (jh outer: half 0 only needs x cols 0:512)
                    for jh in range(2):
                        for oc in range(NCH):
                            sl = slice(jh * 512, (jh + 1) * 512)
                            ps = apsum.tile([P, 512], FP32, tag="aps")
                            for kc in range(NCH):
                                nc.tensor.matmul(
                                    ps,
                                    wk_t[:, kc, oc * P:(oc + 1) * P],
                                    xT_t[:, kc, sl],
                                    start=(kc == 0),
                                    stop=(kc == NCH - 1),
                                )
                            nc.scalar.activation(
                                KT[:, oc, sl], ps, AF.Identity,
                                bias=bias_t[:, 2, oc:oc + 1],
                            )

                    # V [j, hd] -> VA with per-head 65-col slots (ones col)
                    for jc in range(NJC):
                        for mh in range(2):
                            sl = slice(mh * 512, (mh + 1) * 512)
                            vps = apsum.tile([P, 4, 2, D], FP32, tag="aps")
                            for kc in range(NCH):
                                nc.tensor.matmul(
                                    vps,
                                    xT_t[:, kc, jc * P:(jc + 1) * P],
                                    wv_t[:, kc, sl],
                                    start=(kc == 0),
                                    stop=(kc == NCH - 1),
                                )
                            for b in range(2):
                                nc.vector.tensor_copy(
                                    VA[:, jc, 4 * mh:4 * mh + 4, 65 * b:65 * b + D],
                                    vps[:, :, b, :],
                                )
                    nc.vector.memset(VA[:, :, :, D:D + 1], 1.0)
                    nc.vector.memset(VA[:, :, :, 65 + D:65 + D + 1], 1.0)

            # ---------------- Stage B: pair loop ----------------
            nc.sync.dma_start(WPROJ, wproj.rearrange("(c p) f -> p c f", p=P))
            nc.vector.tensor_copy(dmy2[:, 0:2], WPROJ[:, 0, 0:2])
            nc.scalar.copy(dmy2[:, 2:4], WPROJ[:, 0, 0:2])

            cps = sb_ctx.enter_context(tc.tile_pool(name="cps", bufs=2, space="PSUM"))
            DEPTH = max(1, PRO)
            for hp in range(NHP):
                if hp not in s_ts:
                    s_ts[hp] = emit_P(hp)
                if hp + DEPTH < NHP:
                    s_ts[hp + DEPTH] = emit_P(hp + DEPTH)
                emit_CAV(hp, s_ts.pop(hp))
            flush_norm()
            sb_ctx.close()

            # ---------------- Final projection ----------------
            with tc.tile_pool(name="fin", bufs=1) as fin, tc.tile_pool(
                name="fps", bufs=3, space="PSUM"
            ) as fps:
                o_all = fin.tile([P, NIB, DIM], FP32, tag="o_all")
                for ib in range(NIB):
                    isl = slice(ib * P, (ib + 1) * P)
                    for eh in range(2):
                        esl = slice(eh * 512, (eh + 1) * 512)
                        fp = fps.tile([P, 512], FP32, tag="fp")
                        for fc in range(NCH):
                            nc.tensor.matmul(
                                fp, AVT[:, fc, isl], WPROJ[:, fc, esl],
                                start=(fc == 0), stop=False,
                            )
                        nc.tensor.matmul(
                            fp, ones_row, bout_t[:, esl], start=False, stop=True
                        )
                        nc.vector.tensor_copy(o_all[:, ib, esl], fp)
                nc.sync.dma_start(out.rearrange("(ib p) e -> p ib e", p=P), o_all)

    if split_waits:
        _split_multiwait(nc)
    return nc


def _get_nc():
    global _BUILT
    if _BUILT is None:
        _BUILT = _build()
    return _BUILT


def _prep_host(inputs, pos_embedding, full_input, u, v, mask,
               W_kv, b_kv, W_q, b_q, W_pos, b_pos, W_proj, b_proj):
    f32 = np.float32
    W_k = np.ascontiguousarray(W_kv[:, : H * D])
    W_v = np.ascontiguousarray(W_kv[:, H * D:])
    b_k = b_kv[: H * D].astype(f32)
    b_v = b_kv[H * D:].astype(f32)
    bias_qu = (b_q + u.ravel()).astype(f32)
    bias_qv = (b_q + v.ravel()).astype(f32)
    b_out = (b_v @ W_proj + b_proj).astype(f32)

    bias_all = np.stack(
        [bias_qu.reshape(NCH, P), bias_qv.reshape(NCH, P),
         b_k.reshape(NCH, P), np.zeros((NCH, P), f32)], axis=0
    )  # [4, NCH, P]
    bias_all = np.ascontiguousarray(bias_all.transpose(2, 0, 1).reshape(P, 4 * NCH))
    wmats_np = np.concatenate([W_q, W_k, W_v], axis=1).astype(nbf16)
    # R projection is batch-independent: fold it into host prep entirely
    r_np = (pos_embedding[:, 0].astype(f32) @ W_pos.astype(f32)
            + b_pos.astype(f32))  # [FULL, H*D]
    shared = {
        "wmats": wmats_np,
        "rmat": np.ascontiguousarray(r_np.T).astype(nbf16),
        "wproj": W_proj.astype(nbf16),
        "biases": bias_all.astype(f32),
        "bout": b_out.astype(nbf16),
    }
    in_maps = []
    for c in range(BS):
        m = dict(shared)
        m["acts"] = np.concatenate(
            [full_input[:, c].T, inputs[:, c].T], axis=1
        ).astype(nbf16)
        in_maps.append(m)
    return in_maps


def kernel(**inputs):
    nc = _get_nc()
    in_maps = _prep_host(**{k: np.asarray(v) for k, v in inputs.items()})
    res = run_bass_kernel_spmd(nc, in_maps, list(range(BS)))
    out = np.stack([res.results[c]["out"] for c in range(BS)], axis=1)
    return np.ascontiguousarray(out.astype(np.float32))


if __name__ == "__main__":
    nc = _build()
    print("built ok")



# revision 6
# speedup vs baseline: 1.0046x; 1.0046x over previous
"""TransformerXL attention (AttentionXL) Bass kernel for Trainium2, 8 NeuronCores.

Sharding: pure data-parallel over batch (BS=8 -> 1 batch element per core).
All weights replicated per core; no collectives.

Transposed-score pipeline: attention scores live as [key j, query i] so the
attention matrix never needs a PE transpose (v1 spent ~120us/core on 416 of
them and the HAM clock-gate punished the idle gaps they left):

  Host prep:  X^T, Xc^T, W_kv split, bias folds, and the whole batch-
              independent R projection R = pos_emb @ W_pos + b_pos.
  Device, stage A:  KT [hd, j], QuT/QvT [hd, i] (+bias), VA [j, 65-col slots
              per head: V_h | ones] - the ones column makes the AV matmul
              also emit the softmax normalizer Z as PSUM row 64.
  The rel-shift: P [i, m] is written to DRAM with row pitch 1025 and a +1
  pre-pad, which makes S[i, j] = P[i, 511+j-i] one CONTIGUOUS [512, 1024]
  block at offset 512; a single hardware xbar transpose-DMA per head lands
  S^T [j, i] in SBUF.  The pad slot and the sub-diagonal region carry -30000
  poison, so every causally masked position (j - i > 512) reads poison and
  exp()s to zero - no mask op ever touches the score matrix.
  Per head pair (heads 2hp/2hp+1 on PE row-groups 0-63/64-127, emitted
  adjacently so the 64-contraction score matmuls run concurrently; each
  PSUM pair tile is [P, 2, 512] so the two concurrent matmul groups sit in
  different 2KB banks - two groups in ONE bank hang the device):
    C^T [j, i] chunks (trimmed to i >= 128*(jc-4)); DVE adds S^T in PSUM;
    ScalarE exp overwrites S^T in SBUF with E; AV accumulates
    O^T_aug [65, i] = sum_jc VA^T E; 1/Z (DVE reciprocal) is broadcast to 64
    partitions by a tiny ones-column matmul and fused into the PSUM->SBUF
    normalize copy.
  The first 3 pairs' position scores are emitted between the Q and K/V
  projections so their DMA round trips and transposes hide under ~55us of
  stage-A matmuls; the pair loop then software-pipelines 3 deep.
  out[i,e] = AVT^T @ W_proj + b_out (ones-row bias matmul), fp32.
"""

import os
import sys

for _p in (
    "/root/.axon_site",
    "/root/.axon_site/_ro/trn_rl_repo",
    "/root/.axon_site/_ro/pypackages",
    "/opt/trn_rl_repo",
):
    if os.path.isdir(_p) and _p not in sys.path:
        sys.path.append(_p)

import numpy as np
import ml_dtypes

import concourse.bass as bass
import concourse.mybir as mybir
import concourse.tile as tile
from concourse.bass_utils import run_bass_kernel_spmd
from concourse.masks import make_identity

BF16 = mybir.dt.bfloat16
FP32 = mybir.dt.float32
AF = mybir.ActivationFunctionType
ALU = mybir.AluOpType
nbf16 = ml_dtypes.bfloat16

CUR, FULL, BS, DIM, H, D = 512, 1024, 8, 1024, 16, 64
PREV = FULL - CUR
SCALE = 1.0 / D**0.5
P = 128
NIB = CUR // P    # 4 query blocks
NJC = FULL // P   # 8 key chunks
NCH = DIM // P    # 8 dim chunks
NHP = H // 2      # 8 head pairs
PITCH = FULL + 1   # 1025: pad | 1024 score cols; makes the rel-shift contiguous
PITCH2 = 2 * FULL + 1  # 2049: pad | par0 row | par1 row -> ONE transpose per pair
POISON = -30000.0

_BUILT = None


def _split_multiwait(nc):
    """walrus here encodes at most ONE sync wait per TPB instruction
    (NEURON_ISA_TPB_EVENTS has a single wait slot).  Split every
    multi-wait instruction: prepend same-engine NoOps carrying the
    extra waits, keep the last wait on the instruction itself."""
    n_split = 0
    for fn in nc.m.functions:
        for blk in fn.blocks:
            insts = list(blk.instructions)
            out = []
            for ins in insts:
                si = ins.sync_info
                if si is not None and si.on_wait and len(si.on_wait) > 1:
                    waits = list(si.on_wait)
                    for w in waits[:-1]:
                        nop = mybir.InstNoOp(
                            name=f"{ins.name}-ws{n_split}",
                            engine=ins.engine,
                            sync_info=mybir.SyncInfo(on_wait=[w], on_update=[]),
                            text_hint="waitsplit",
                        )
                        out.append(nop)
                        n_split += 1
                    ins.sync_info = mybir.SyncInfo(
                        on_wait=[waits[-1]],
                        on_update=list(si.on_update or []),
                    )
                out.append(ins)
            blk.instructions = out
    return n_split


def _build(split_waits=True):
    nc = bass.Bass()

    # acts: [X^T | Xc^T | Pos^T] cols; wmats: [W_q | W_pos | W_k | W_v] cols
    acts = nc.declare_dram_parameter("acts", [DIM, FULL + CUR], BF16, isOutput=False)
    wmats = nc.declare_dram_parameter("wmats", [DIM, 3 * DIM], BF16, isOutput=False)
    rmat = nc.declare_dram_parameter("rmat", [DIM, FULL], BF16, isOutput=False)
    wproj = nc.declare_dram_parameter("wproj", [DIM, DIM], BF16, isOutput=False)
    # biases pre-laid-out on host: [p, 4*NCH] = qu | qv | k | pos chunks
    biases = nc.declare_dram_parameter("biases", [P, 4 * NCH], FP32, isOutput=False)
    bout = nc.declare_dram_parameter("bout", [DIM], BF16, isOutput=False)
    out = nc.declare_dram_parameter("out", [CUR, DIM], FP32, isOutput=True)

    with tile.TileContext(nc) as tc:
        from contextlib import ExitStack

        with ExitStack() as ctx:
            persist = ctx.enter_context(tc.tile_pool(name="persist", bufs=1))

            KT = persist.tile([P, NCH, FULL], BF16, tag="KT")
            RT = persist.tile([P, NCH, FULL], BF16, tag="RT")
            QuT = persist.tile([P, NCH, CUR], BF16, tag="QuT")
            QvT = persist.tile([P, NCH, CUR], BF16, tag="QvT")
            # per head-pair: [Va(64) | 1 | Vb(64) | 1] -> 130 cols
            VA = persist.tile([P, NJC, NHP, 130], BF16, tag="VA")
            AVT = persist.tile([P, NCH, CUR], BF16, tag="AVT")
            ones_row = persist.tile([P, P], BF16, tag="ones_row")
            ones1 = persist.tile([1, D], BF16, tag="ones1")
            bout_t = persist.tile([P, DIM], BF16, tag="bout_t")
            bias_t = persist.tile([P, 4, NCH], FP32, tag="bias_t")  # qu|qv|k|pos

            ident = persist.tile([P, P], BF16, tag="ident")
            make_identity(nc, ident)
            poison_reg = nc.gpsimd.to_reg(POISON)
            nc.vector.memset(ones_row, 0.0)
            nc.vector.memset(ones_row[0:1, :], 1.0)
            nc.vector.memset(ones1, 1.0)
            nc.vector.memset(bout_t, 0.0)
            nc.sync.dma_start(bout_t[0:1, :], bout[None, :])
            nc.sync.dma_start(bias_t, biases.rearrange("p (b c) -> p b c", b=4))

            # ---------------- Stage A + pipelined stage B ----------------
            # PSUM phasing: apsum(4) + pps(4) during projections/prologue,
            # then apsum closes and cps(4) opens for the pair loop.
            late = ctx.enter_context(tc.tile_pool(name="late", bufs=1))
            WPROJ = late.tile([P, NCH, DIM], BF16, tag="WPROJ")
            dmy2 = late.tile([P, 4], FP32, tag="dmy2")

            sb_ctx = ctx.enter_context(ExitStack())
            p_pool = sb_ctx.enter_context(tc.tile_pool(name="p_pool", bufs=2))
            s_pool = sb_ctx.enter_context(tc.tile_pool(name="s_pool", bufs=3))
            rz_pool = sb_ctx.enter_context(tc.tile_pool(name="rz_pool", bufs=2))
            nrm_pool = sb_ctx.enter_context(tc.tile_pool(name="nrm_pool", bufs=4))
            dram = sb_ctx.enter_context(tc.tile_pool(name="dram", bufs=3, space="DRAM"))
            pps = sb_ctx.enter_context(tc.tile_pool(name="pps", bufs=2, space="PSUM"))
            cps = None  # opened after stage A's psum pool closes

            def emit_P(hp):
                """Position scores for head pair hp -> sheared DRAM ->
                per-parity transpose-DMAs back as S^T [par, j, i] (poison
                masking), issued on sync+scalar HWDGE queues to overlap.
                Pair-row layout [pad | par0 row | par1 row] at pitch 2049
                keeps the rel-shift contiguous across BOTH heads."""
                pd = dram.tile([CUR, PITCH2], BF16, tag="pd")
                s_t = s_pool.tile([P, 2, NJC, CUR], BF16, tag="s_t")
                for ib in range(NIB):
                    isl = slice(ib * P, (ib + 1) * P)
                    pib = p_pool.tile([P, PITCH2], BF16, tag="pib")
                    if ib < 3:
                        # pad + m < 128 of each parity: poison (masked reads)
                        nc.vector.memset(pib[:, 0:129], POISON)
                        nc.vector.memset(pib[:, 1025:1153], POISON)
                        mranges = [(128, 576), (576, 1024)]
                    else:
                        nc.vector.memset(pib[:, 0:1], POISON)
                        mranges = [(0, 512), (512, 1024)]
                    pibap = pib[:, :]
                    for q, (m0, m1) in enumerate(mranges):
                        w = m1 - m0
                        # [P, 2, 512]: each parity in its own PSUM bank --
                        # two concurrent matmul groups in ONE bank hang the
                        # device (probed), bank-split pairs are fine
                        pp = pps.tile([P, 2, 512], FP32, tag="pp")
                        for b in range(2):
                            rs = slice(b * D, (b + 1) * D)
                            nc.tensor.matmul(
                                pp[:, b, :w],
                                QvT[rs, hp, isl],
                                RT[rs, hp, m0:m1],
                                start=True, stop=True,
                            )
                        dst = bass.AP(tensor=pibap.tensor,
                                      offset=pibap.offset + 1 + m0,
                                      ap=[pibap.ap[0], [1024, 2], [1, w]])
                        ceng = nc.scalar.copy if q % 2 == 0 else nc.vector.tensor_copy
                        ceng(dst, pp[:, :, :w])
                    if ib == 3:
                        # sub-diagonal triangle m < 511-r for rows 384+u
                        tri = bass.AP(tensor=pibap.tensor,
                                      offset=pibap.offset + 1,
                                      ap=[pibap.ap[0], [1024, 2], [1, 128]])
                        nc.gpsimd.affine_select(
                            out=tri, in_=tri,
                            compare_op=ALU.is_ge,
                            fill=poison_reg,
                            base=-127,
                            channel_multiplier=1,
                            pattern=[[0, 2], [1, 128]],
                        )
                    dest = bass.AP(
                        tensor=pd.tensor,
                        offset=pd.offset + ib * P * PITCH2,
                        ap=[[PITCH2, P], [1, PITCH2]],
                    )
                    nc.sync.dma_start(dest, pib)
                src = bass.AP(
                    tensor=pd.tensor,
                    offset=pd.offset + 512,
                    ap=[[2048, 512], [1, 2048]],
                )
                nc.sync.dma_start(s_t, src, transpose=True)
                return s_t

            pending_norm = []

            def emit_CAV(hp, s_t):
                """Content scores + softmax + AV for head pair hp.
                E (exp) overwrites s_t in place.  The previous pair's deferred
                normalize flushes between the chunk and AV phases."""
                for jc in range(NJC):
                    jsl = slice(jc * P, (jc + 1) * P)
                    ilo = max(0, jc - 4) * P
                    w = CUR - ilo
                    cp = cps.tile([P, 2, 512], FP32, tag="cp")
                    for b in range(2):
                        rs = slice(b * D, (b + 1) * D)
                        nc.tensor.matmul(
                            cp[:, b, :w],
                            KT[rs, hp, jsl],
                            QuT[rs, hp, ilo:],
                            start=True, stop=False,
                        )
                    # accumulate S^T on the PE (identity matmul): frees the
                    # DVE add AND keeps TensorE busy enough to hold K=8/8
                    for b in range(2):
                        nc.tensor.matmul(
                            cp[:, b, :w],
                            ident,
                            s_t[:, b, jc, ilo:],
                            start=False, stop=True,
                        )
                    nc.scalar.activation(
                        s_t[:, :, jc, ilo:], cp[:, :, :w], AF.Exp, scale=SCALE
                    )
                flush_norm()
                avs = []
                for b in range(2):
                    av_t = cps.tile([P, 2, 512], FP32, tag="cp")
                    for jc in range(NJC):
                        ilo = max(0, jc - 4) * P
                        nc.tensor.matmul(
                            av_t[0:D + 1, 0, ilo:],
                            VA[:, jc, hp, 65 * b:65 * b + D + 1],
                            s_t[:, b, jc, ilo:],
                            start=(jc == 0),
                            stop=(jc == NJC - 1),
                        )
                    avs.append(av_t)
                # ALL PSUM-releasing copies go on DVE BEFORE the slow
                # reciprocals: the next pair's content matmuls wait on a cps
                # buffer, and a 3.3us reciprocal queued between the copies
                # stalled the PE 4.4us per pair (trace: RECIPROCAL slices
                # bracketing the av_sb CASTs).  Copies first -> av_t frees
                # ~1.4us after the AV matmuls; recips then overlap the next
                # pair's content phase.
                work = []
                for b in range(2):
                    rs = slice(b * D, (b + 1) * D)
                    av_sb = nrm_pool.tile([P, CUR], BF16, tag="av_sb")
                    nc.vector.tensor_copy(av_sb[rs, :], avs[b][0:D, 0, :])
                    zrow = rz_pool.tile([1, CUR], BF16, tag="zrow")
                    nc.vector.tensor_copy(zrow, avs[b][D:D + 1, 0, :])
                    work.append((b, av_sb, zrow))
                # the PE-side broadcast matmul is DEFERRED until the next
                # pair's chunk phase so it never head-of-line-blocks the PE
                # queue waiting for the reciprocal (16 such stalls before)
                for b, av_sb, zrow in work:
                    rzh = nrm_pool.tile([1, CUR], BF16, tag="rzh")
                    with nc.allow_low_precision(reason="1/Z bf16; v1-class precision"):
                        nc.vector.reciprocal(rzh, zrow)
                    pending_norm.append((hp, b, av_sb, rzh))

            def flush_norm():
                while pending_norm:
                    hp, b, av_sb, rzh = pending_norm.pop(0)
                    rs = slice(b * D, (b + 1) * D)
                    bt = cps.tile([P, 2, 512], FP32, tag="cp")
                    nc.tensor.matmul(bt[rs, 1, :], ones1, rzh,
                                     start=True, stop=True, tile_position=(0, b * D))
                    bc_sb = rz_pool.tile([P, CUR], BF16, tag="bc_sb")
                    nc.scalar.copy(bc_sb[rs, :], bt[rs, 1, :])
                    nc.vector.tensor_tensor(
                        AVT[rs, hp, :], av_sb[rs, :], bc_sb[rs, :], ALU.mult
                    )

            with tc.tile_pool(name="apsum", bufs=4, space="PSUM") as apsum:
                # -- phase 1: Q projection (xc, wq) --
                with tc.tile_pool(name="ain1", bufs=1) as ain1:
                    xcw = ain1.tile([P, NCH, CUR + DIM], BF16, tag="xcw")
                    acts_r = acts.rearrange("(c p) f -> p c f", p=P)
                    wmats_r = wmats.rearrange("(c p) f -> p c f", p=P)
                    rmat_r = rmat.rearrange("(c p) m -> p c m", p=P)
                    # head trim: first matmul needs xcT + wq oc-chunk 0 only.
                    # Split wq so oc 0-3 land first, and defer most of RT
                    # (only prologue pairs 0-1 need it early) -> Q-proj
                    # starts after ~3.5MB instead of 5MB of loads.
                    nc.sync.dma_start(xcw[:, :, 0:CUR], acts_r[:, :, FULL:FULL + CUR])
                    nc.sync.dma_start(xcw[:, :, CUR:CUR + 512],
                                      wmats_r[:, :, 0:512])
                    nc.sync.dma_start(xcw[:, :, CUR + 512:],
                                      wmats_r[:, :, 512:DIM])
                    nc.sync.dma_start(RT[:, 0:2, :], rmat_r[:, 0:2, :])
                    xcT_t = xcw[:, :, 0:CUR]
                    wq_t = xcw[:, :, CUR:]

                    dmy = ain1.tile([P, 24], FP32, tag="dmy")
                    col = [0]
                    def _observe(eng):
                        for srcap in (xcT_t[:, 0, 0:2], wq_t[:, 0, 0:2],
                                      RT[:, 0, 0:2], bias_t[:, 0, 0:2],
                                      bout_t[:, 0:2]):
                            eng(dmy[:, col[0]: col[0] + 2], srcap)
                            col[0] = (col[0] + 2) % 24
                    _observe(nc.vector.tensor_copy)
                    _observe(nc.scalar.copy)

                    # Q^T [hd, i] then QuT/QvT with per-partition bias
                    for oc in range(NCH):
                        ps = apsum.tile([P, CUR], FP32, tag="aps")
                        for kc in range(NCH):
                            nc.tensor.matmul(
                                ps,
                                wq_t[:, kc, oc * P:(oc + 1) * P],
                                xcT_t[:, kc, :],
                                start=(kc == 0),
                                stop=(kc == NCH - 1),
                            )
                        nc.scalar.activation(
                            QuT[:, oc, :], ps, AF.Identity, bias=bias_t[:, 0, oc:oc + 1]
                        )
                        nc.scalar.activation(
                            QvT[:, oc, :], ps, AF.Identity, bias=bias_t[:, 1, oc:oc + 1]
                        )
                    nc.sync.dma_start(RT[:, 2:, :], rmat_r[:, 2:, :])

                    # -- prologue: first PRO head pairs' position scores; their
                    # DMA round trips + transposes hide under the K/V matmuls
                    PRO = int(os.environ.get("V2_PRO", "2"))
                    s_ts = {}
                    for hp in range(PRO):
                        s_ts[hp] = emit_P(hp)

                # -- phase 2: K and V projections (x, wk, wv) --
                with tc.tile_pool(name="ain2", bufs=1) as ain2:
                    xw = ain2.tile([P, NCH, FULL + 2 * DIM], BF16, tag="xw")
                    # x split into j-halves so the K projection's first half
                    # starts after 4MB of loads instead of 6MB
                    nc.sync.dma_start(xw[:, :, 0:512], acts_r[:, :, 0:512])
                    nc.sync.dma_start(xw[:, :, FULL:FULL + DIM],
                                      wmats_r[:, :, DIM:2 * DIM])
                    nc.sync.dma_start(xw[:, :, 512:FULL], acts_r[:, :, 512:FULL])
                    nc.sync.dma_start(xw[:, :, FULL + DIM:],
                                      wmats_r[:, :, 2 * DIM:3 * DIM])
                    xT_t = xw[:, :, 0:FULL]
                    wk_t = xw[:, :, FULL:FULL + DIM]
                    wv_t = xw[:, :, FULL + DIM:]
                    dmy3 = ain2.tile([P, 8], FP32, tag="dmy3")
                    for eng in (nc.vector.tensor_copy, nc.scalar.copy):
                        eng(dmy3[:, 0:2], xT_t[:, 0, 0:2])
                        eng(dmy3[:, 2:4], wk_t[:, 0, 0:2])
                        eng(dmy3[:, 4:6], wv_t[:, 0, 0:2])

                    # K^T [hd, j]  (jh outer: half 0 only needs x cols 0:512)
                    for jh in range(2):
                        for oc in range(NCH):
                            sl = slice(jh * 512, (jh + 1) * 512)
                            ps = apsum.tile([P, 512], FP32, tag="aps")
                            for kc in range(NCH):
                                nc.tensor.matmul(
                                    ps,
                                    wk_t[:, kc, oc * P:(oc + 1) * P],
                                    xT_t[:, kc, sl],
                                    start=(kc == 0),
                                    stop=(kc == NCH - 1),
                                )
                            nc.scalar.activation(
                                KT[:, oc, sl], ps, AF.Identity,
                                bias=bias_t[:, 2, oc:oc + 1],
                            )

                    # V [j, hd] -> VA with per-head 65-col slots (ones col)
                    for jc in range(NJC):
                        for mh in range(2):
                            sl = slice(mh * 512, (mh + 1) * 512)
                            vps = apsum.tile([P, 4, 2, D], FP32, tag="aps")
                            for kc in range(NCH):
                                nc.tensor.matmul(
                                    vps,
                                    xT_t[:, kc, jc * P:(jc + 1) * P],
                                    wv_t[:, kc, sl],
                                    start=(kc == 0),
                                    stop=(kc == NCH - 1),
                                )
                            for b in range(2):
                                nc.vector.tensor_copy(
                                    VA[:, jc, 4 * mh:4 * mh + 4, 65 * b:65 * b + D],
                                    vps[:, :, b, :],
                                )
                    nc.vector.memset(VA[:, :, :, D:D + 1], 1.0)
                    nc.vector.memset(VA[:, :, :, 65 + D:65 + D + 1], 1.0)

            # ---------------- Stage B: pair loop ----------------
            nc.sync.dma_start(WPROJ, wproj.rearrange("(c p) f -> p c f", p=P))
            nc.vector.tensor_copy(dmy2[:, 0:2], WPROJ[:, 0, 0:2])
            nc.scalar.copy(dmy2[:, 2:4], WPROJ[:, 0, 0:2])

            cps = sb_ctx.enter_context(tc.tile_pool(name="cps", bufs=2, space="PSUM"))
            DEPTH = max(1, PRO)
            for hp in range(NHP):
                if hp not in s_ts:
                    s_ts[hp] = emit_P(hp)
                if hp + DEPTH < NHP:
                    s_ts[hp + DEPTH] = emit_P(hp + DEPTH)
                emit_CAV(hp, s_ts.pop(hp))
            flush_norm()
            sb_ctx.close()

            # ---------------- Final projection ----------------
            with tc.tile_pool(name="fin", bufs=1) as fin, tc.tile_pool(
                name="fps", bufs=3, space="PSUM"
            ) as fps:
                o_all = fin.tile([P, NIB, DIM], FP32, tag="o_all")
                out_r = out.rearrange("(ib p) e -> p ib e", p=P)
                for ib in range(NIB):
                    isl = slice(ib * P, (ib + 1) * P)
                    for eh in range(2):
                        esl = slice(eh * 512, (eh + 1) * 512)
                        fp = fps.tile([P, 512], FP32, tag="fp")
                        for fc in range(NCH):
                            nc.tensor.matmul(
                                fp, AVT[:, fc, isl], WPROJ[:, fc, esl],
                                start=(fc == 0), stop=False,
                            )
                        nc.tensor.matmul(
                            fp, ones_row, bout_t[:, esl], start=False, stop=True
                        )
                        # alternate evac engines so the two copies of one ib
                        # run concurrently; per-ib DMA overlaps the next ib
                        ceng = nc.vector.tensor_copy if eh == 0 else nc.scalar.copy
                        ceng(o_all[:, ib, esl], fp)
                    nc.sync.dma_start(out_r[:, ib:ib + 1, :], o_all[:, ib:ib + 1, :])

    if split_waits:
        _split_multiwait(nc)
    return nc


def _get_nc():
    global _BUILT
    if _BUILT is None:
        _BUILT = _build()
    return _BUILT


def _prep_host(inputs, pos_embedding, full_input, u, v, mask,
               W_kv, b_kv, W_q, b_q, W_pos, b_pos, W_proj, b_proj):
    f32 = np.float32
    W_k = np.ascontiguousarray(W_kv[:, : H * D])
    W_v = np.ascontiguousarray(W_kv[:, H * D:])
    b_k = b_kv[: H * D].astype(f32)
    b_v = b_kv[H * D:].astype(f32)
    bias_qu = (b_q + u.ravel()).astype(f32)
    bias_qv = (b_q + v.ravel()).astype(f32)
    b_out = (b_v @ W_proj + b_proj).astype(f32)

    bias_all = np.stack(
        [bias_qu.reshape(NCH, P), bias_qv.reshape(NCH, P),
         b_k.reshape(NCH, P), np.zeros((NCH, P), f32)], axis=0
    )  # [4, NCH, P]
    bias_all = np.ascontiguousarray(bias_all.transpose(2, 0, 1).reshape(P, 4 * NCH))
    wmats_np = np.concatenate([W_q, W_k, W_v], axis=1).astype(nbf16)
    # R projection is batch-independent: fold it into host prep entirely
    r_np = (pos_embedding[:, 0].astype(f32) @ W_pos.astype(f32)
            + b_pos.astype(f32))  # [FULL, H*D]
    shared = {
        "wmats": wmats_np,
        "rmat": np.ascontiguousarray(r_np.T).astype(nbf16),
        "wproj": W_proj.astype(nbf16),
        "biases": bias_all.astype(f32),
        "bout": b_out.astype(nbf16),
    }
    in_maps = []
    for c in range(BS):
        m = dict(shared)
        m["acts"] = np.concatenate(
            [full_input[:, c].T, inputs[:, c].T], axis=1
        ).astype(nbf16)
        in_maps.append(m)
    return in_maps


def kernel(**inputs):
    nc = _get_nc()
    in_maps = _prep_host(**{k: np.asarray(v) for k, v in inputs.items()})
    res = run_bass_kernel_spmd(nc, in_maps, list(range(BS)))
    out = np.stack([res.results[c]["out"] for c in range(BS)], axis=1)
    return np.ascontiguousarray(out.astype(np.float32))


if __name__ == "__main__":
    nc = _build()
    print("built ok")

